# revision 8
# baseline (speedup 1.0000x reference)
"""HRR binding self-attention kernel for 8 trn2 NeuronCores.

Math: out = irfft(c * rfft(x) * cumsum_s(rfft(x))) @ w_out.T  with c = queries*keyvalues.
Since rfft is linear, cumsum commutes with it: only ONE forward DFT of x is needed;
the causal prefix sum runs in the frequency domain.  Two further fusions:
  * irfft followed by the output Linear is one linear map:  out = qv^T (G @ w_out.T),
    precomputed on host as M (packed-spectrum x model_dims).
  * the real per-frequency filter c is diagonal in the packed spectrum, so it folds
    into M as a row scale:  M_c = diag(c_packed) G w_out^T.
So the device does: DFT (matmul), causal prefix sum + transpose (triangular matmul),
complex pointwise multiply (DVE), and ONE output matmul with M_c.

Sharding: 8 shards = (batch b in 0..3) x (seq half h in 0..1), 2048 tokens each.
The h=1 shards get the first half's contribution as an initial carry, computed on
host as rfft(x[b, :2048].sum(0)) (O(B*D log D) -- negligible).

Packed real spectrum (2048 rows): rows 0..1024 = Re[0..1024], rows 1025..2047 =
Im[1..1023].  Row 1024 (Nyquist, purely real) rides in the Im-block's first slot
(chunk 8, partition 0); complex multiplies pair chunk i with chunk 8+i on equal
partitions, with a 2-row fixup for the DC/Nyquist slots.

Per-core pipeline (all matmuls bf16 with fp32 PSUM accumulate), per 128-token tile:
  DFT (x^T tile @ CS) -> token-major spectrum; triangular matmul (spec @ [U|I]) =
  within-tile cumsum AND transpose to freq-major in one shot; carry added as
  per-partition ACT bias; 6 wide DVE ops do the complex multiply -> qv (SBUF only,
  never spilled); the PREVIOUS tile's output matmul (qv_chunk^T @ M_c) is emitted
  interleaved with this tile's triangular stage so the PE never idles waiting on
  the scalar-engine eviction chain.
"""

import sys

sys.path.insert(0, "/opt/trn_rl_repo")

import numpy as np
import ml_dtypes

import concourse.bass as bass
import concourse.bacc as bacc
import concourse.mybir as mybir
from concourse.tile import TileContext
from concourse.bass_utils import run_bass_kernel_spmd

BF16 = mybir.dt.bfloat16
F32 = mybir.dt.float32
AF = mybir.ActivationFunctionType

P = 128
D = 2048  # model dims
T = 2048  # tokens per shard
ND = D // P  # 16 d-chunks
NPF = 16  # packed-frequency chunks
NT = T // P  # 16 token tiles
NB = 4  # batch
NS = 4096  # full seq

bf16 = ml_dtypes.bfloat16

_CACHE = {}


def _build_nc(reps: int = 1):
    nc = bacc.Bacc("TRN2", target_bir_lowering=False, debug=False, num_devices=8)
    xT = nc.dram_tensor("xT", [NT, P, ND, P], BF16, kind="ExternalInput")
    CS = nc.dram_tensor("CS", [P, ND, D], BF16, kind="ExternalInput")
    M = nc.dram_tensor("M", [P, NPF, D], BF16, kind="ExternalInput")
    UI = nc.dram_tensor("UI", [P, 2 * P], BF16, kind="ExternalInput")
    C0 = nc.dram_tensor("C0", [P, NPF], F32, kind="ExternalInput")
    out = nc.dram_tensor("out", [T, D], F32, kind="ExternalOutput")

    with TileContext(nc) as tc:
        with tc.tile_pool(name="misc", bufs=1) as misc:
            ui_sb = misc.tile([P, 2 * P], BF16)
            nc.sync.dma_start(ui_sb[:], UI[:])
            c0_sb = misc.tile([P, NPF], F32)
            nc.sync.dma_start(c0_sb[:], C0[:])
            # weights stay resident across repeat-loop iterations
            cs_sb = misc.tile([P, ND, D], BF16)
            nc.sync.dma_start(cs_sb[:], CS[:])
            m_sb = misc.tile([P, NPF, D], BF16)
            nc.sync.dma_start(m_sb[:], M[:])

            import contextlib

            loop_ctx = (
                tc.For_i(0, reps, 1) if reps > 1 else contextlib.nullcontext()
            )
            with loop_ctx:
                _body(nc, tc, ui_sb, c0_sb, cs_sb, m_sb, xT, out)
    nc.finalize()
    return nc


def _body(nc, tc, ui_sb, c0_sb, cs_sb, m_sb, xT, out):
    with (
        tc.tile_pool(name="xt", bufs=3) as xpool,
        tc.tile_pool(name="xh", bufs=2) as xhpool,
        tc.tile_pool(name="sq", bufs=3) as sqpool,
        tc.tile_pool(name="qq", bufs=2) as qpool,
        tc.tile_pool(name="tmp", bufs=1) as tpool,
        tc.tile_pool(name="qvp", bufs=3) as qvpool,
        tc.tile_pool(name="osb", bufs=4) as opool,
        tc.tile_pool(name="psA", bufs=4, space="PSUM") as psumA,
        tc.tile_pool(name="psT", bufs=2, space="PSUM") as psumT,
        tc.tile_pool(name="psB", bufs=2, space="PSUM") as psumB,
    ):
        def emit_B_gen(qv_t, t):
            """Output matmul for one 128-token tile, yielded stepwise so it can
            be interleaved into PE gaps of the next tile's front end."""
            for e in range(4):
                psb = psumB.tile([P, 512], F32, tag="psB")
                for pf in range(NPF):
                    nc.tensor.matmul(
                        psb[:],
                        qv_t[:, pf, :],
                        m_sb[:, pf, e * 512 : (e + 1) * 512],
                        start=(pf == 0),
                        stop=(pf == NPF - 1),
                    )
                    yield
                ob = opool.tile([P, 512], F32, tag="osb")
                if e % 2 == 0:
                    nc.vector.tensor_copy(ob[:], psb[:])
                else:
                    nc.scalar.copy(ob[:], psb[:])
                nc.sync.dma_start(
                    out[t * P : (t + 1) * P, e * 512 : (e + 1) * 512], ob[:]
                )
                yield

        def adv(gen, n):
            if gen is None:
                return
            for _ in range(n):
                if next(gen, "done") == "done":
                    return

        carry_prev = None
        bgen = None
        for t in range(NT):
            xt = xpool.tile([P, ND, P], BF16, tag="xt")
            nc.sync.dma_start(xt[:], xT[t])
            xh = xhpool.tile([P, D], BF16, tag="xh")
            psA4 = [
                psumA.tile([P, 512], F32, tag="psA", name=f"psA{_fq}")
                for _fq in range(4)
            ]
            for d in range(ND):
                for fq in range(4):
                    nc.tensor.matmul(
                        psA4[fq][:],
                        xt[:, d, :],
                        cs_sb[:, d, fq * 512 : (fq + 1) * 512],
                        start=(d == 0),
                        stop=(d == ND - 1),
                    )
            for fq in range(4):
                if fq % 2 == 0:
                    nc.vector.tensor_copy(
                        xh[:, fq * 512 : (fq + 1) * 512], psA4[fq][:]
                    )
                else:
                    nc.scalar.copy(
                        xh[:, fq * 512 : (fq + 1) * 512], psA4[fq][:]
                    )

            # previous tile's output matmul fills the PE gap while this tile's
            # spectrum drains from PSUM
            adv(bgen, 8)

            S_sb = sqpool.tile([P, NPF, P], BF16, tag="S")
            Q_sb = qpool.tile([P, NPF, P], BF16, tag="Q")
            for g in range(8):
                pst = psumT.tile([P, 2, 2 * P], F32, tag="psT")
                for i in range(2):
                    pf = 2 * g + i
                    nc.tensor.matmul(
                        pst[:, i, :],
                        xh[:, pf * P : (pf + 1) * P],
                        ui_sb[:],
                        start=True,
                        stop=True,
                    )
                for i in range(2):
                    pf = 2 * g + i
                    carry_ap = (
                        c0_sb[:, pf : pf + 1]
                        if t == 0
                        else carry_prev[:, pf, P - 1 : P]
                    )
                    nc.scalar.activation(
                        S_sb[:, pf, :], pst[:, i, 0:P], AF.Identity, bias=carry_ap
                    )
                    nc.vector.tensor_copy(Q_sb[:, pf, :], pst[:, i, P : 2 * P])
                adv(bgen, 3)
            carry_prev = S_sb

            qv = qvpool.tile([P, NPF, P], BF16, tag="qv")
            t1 = tpool.tile([P, 8, P], F32, tag="t1")
            t2 = tpool.tile([P, 8, P], F32, tag="t2")
            nc.vector.tensor_mul(t1[:], Q_sb[:, 0:8, :], S_sb[:, 0:8, :])
            nc.vector.tensor_mul(t2[:], Q_sb[:, 8:16, :], S_sb[:, 8:16, :])
            nc.vector.tensor_sub(qv[:, 0:8, :], t1[:], t2[:])
            t3 = tpool.tile([P, 8, P], F32, tag="t1")
            t4 = tpool.tile([P, 8, P], F32, tag="t2")
            nc.vector.tensor_mul(t3[:], Q_sb[:, 0:8, :], S_sb[:, 8:16, :])
            nc.vector.tensor_mul(t4[:], Q_sb[:, 8:16, :], S_sb[:, 0:8, :])
            nc.vector.tensor_add(qv[:, 8:16, :], t3[:], t4[:])
            # DC (chunk 0 row 0) and Nyquist (chunk 8 row 0) are purely real
            nc.vector.tensor_mul(qv[0:1, 0, :], Q_sb[0:1, 0, :], S_sb[0:1, 0, :])
            nc.vector.tensor_mul(qv[0:1, 8, :], Q_sb[0:1, 8, :], S_sb[0:1, 8, :])

            # drain the rest of the previous tile's output matmul
            adv(bgen, 100)
            bgen = emit_B_gen(qv, t)

        adv(bgen, 100)


def _chunked(m):
    """[rows, cols] -> [P, rows//P, cols] with row r at [r % P, r // P]."""
    r, c = m.shape
    return np.ascontiguousarray(m.reshape(r // P, P, c).transpose(1, 0, 2))


def _pack_spec(re, im):
    """re[1025], im[1025] -> packed [2048]: re[0..1024] then im[1..1023]."""
    return np.concatenate([re, im[1:1024]])


def _constants():
    if "consts" in _CACHE:
        return _CACHE["consts"]
    d = np.arange(D, dtype=np.float64)
    f = np.arange(D // 2 + 1, dtype=np.float64)
    ang = 2.0 * np.pi / D * np.outer(d, f)  # [D, 1025]
    cos, sin = np.cos(ang), np.sin(ang)
    CSf = np.concatenate([cos, -sin[:, 1:1024]], axis=1)  # [D, D]
    alpha = np.full(1025, 2.0)
    alpha[0] = alpha[1024] = 1.0
    Gf = np.concatenate(
        [(alpha[:, None] * cos.T) / D, (-2.0 * sin[:, 1:1024].T) / D], axis=0
    )  # [D packed, D]
    U = np.triu(np.ones((P, P)))
    UI = np.concatenate([U, np.eye(P)], axis=1)
    consts = {
        "CS": _chunked(CSf.astype(np.float32)).astype(bf16),
        "Gf32": Gf.astype(np.float32),
        "UI": UI.astype(bf16),
    }
    _CACHE["consts"] = consts
    return consts


def prepare_in_maps(x, queries, keyvalues, w_out):
    x = np.asarray(x, dtype=np.float32)
    queries = np.asarray(queries, dtype=np.float32)
    keyvalues = np.asarray(keyvalues, dtype=np.float32)
    w_out = np.asarray(w_out, dtype=np.float32)
    consts = _constants()

    c = (queries * keyvalues).reshape(-1)  # [1025]
    cpk = _pack_spec(c, c).astype(np.float32)  # [2048]
    # irfft + output Linear + c-filter as ONE matrix: M = diag(c_pk) G w_out^T
    Mfull = (consts["Gf32"] * cpk[:, None]) @ np.ascontiguousarray(w_out.T)
    Mc = _chunked(Mfull).astype(bf16)

    in_maps = []
    shards = []
    for b in range(NB):
        for h in range(2):
            shards.append((b, h))
            xs = x[b, h * T : (h + 1) * T]  # [T, D]
            xT3 = _chunked(np.ascontiguousarray(xs.T))  # [P, ND, T]
            xTc = np.ascontiguousarray(
                xT3.reshape(P, ND, NT, P).transpose(2, 0, 1, 3)
            ).astype(bf16)
            if h == 0:
                c0 = np.zeros((P, NPF), np.float32)
            else:
                F = np.fft.rfft(x[b, :T].sum(axis=0).astype(np.float64))
                c0 = _chunked(
                    _pack_spec(F.real, F.imag).astype(np.float32)[:, None]
                )[:, :, 0]
            in_maps.append(
                {
                    "xT": xTc,
                    "CS": consts["CS"],
                    "M": Mc,
                    "UI": consts["UI"],
                    "C0": np.ascontiguousarray(c0),
                }
            )
    return in_maps, shards


def kernel(x, queries, keyvalues, w_out):
    if "nc" not in _CACHE:
        _CACHE["nc"] = _build_nc()
    nc = _CACHE["nc"]
    in_maps, shards = prepare_in_maps(x, queries, keyvalues, w_out)
    res = run_bass_kernel_spmd(nc, in_maps, core_ids=list(range(8)))
    y = np.empty((NB, NS, D), np.float32)
    for i, (b, h) in enumerate(shards):
        y[b, h * T : (h + 1) * T] = res.results[i]["out"]
    return y


# revision 10
# speedup vs baseline: 1.0487x; 1.0487x over previous
"""HRR binding self-attention kernel for 8 trn2 NeuronCores.

Math: out = irfft(c * rfft(x) * cumsum_s(rfft(x))) @ w_out.T  with c = queries*keyvalues.
Since rfft is linear, cumsum commutes with it: only ONE forward DFT of x is needed;
the causal prefix sum runs in the frequency domain.  Two further fusions:
  * irfft followed by the output Linear is one linear map:  out = qv^T (G @ w_out.T),
    precomputed on host as M (packed-spectrum x model_dims).
  * the real per-frequency filter c is diagonal in the packed spectrum, so it folds
    into M as a row scale:  M_c = diag(c_packed) G w_out^T.
So the device does: DFT (matmul, emitted FREQ-major so the spectrum lands in PSUM
with frequency on partitions), causal prefix sum via the DVE's tensor_tensor_scan
(per-partition recurrence along tokens, carry chained through `initial`), complex
pointwise multiply reading Q directly from PSUM, and ONE output matmul with M_c.
No transpose stage, no PSUM->SBUF spectrum eviction, and only ~24 DMAs/iteration.

Sharding: 8 shards = (batch b in 0..3) x (seq half h in 0..1), 2048 tokens each.
The h=1 shards get the first half's contribution as an initial carry, computed on
host as rfft(x[b, :2048].sum(0)) (O(B*D log D) -- negligible).

Packed real spectrum (2048 rows): rows 0..1024 = Re[0..1024], rows 1025..2047 =
Im[1..1023].  Row 1024 (Nyquist, purely real) rides in the Im-block's first slot
(chunk 8, partition 0); complex multiplies pair chunk i with chunk 8+i on equal
partitions, with a 2-row fixup for the DC/Nyquist slots.

Per-core pipeline over 256-token slabs (all matmuls bf16, fp32 PSUM):
  4 DFT passes per slab, each producing 4 freq-chunks {2p, 2p+1, 2p+8, 2p+9}
  (a Re pair and its Im partner pair) into 2 PSUM banks; scan -> S; 6 DVE ops
  -> qv;  the PREVIOUS slab's output matmul (qv chunk^T @ M_c) is interleaved
  between DFT passes so the PE never idles.
"""

import sys

sys.path.insert(0, "/opt/trn_rl_repo")

import numpy as np
import ml_dtypes

import concourse.bass as bass
import concourse.bacc as bacc
import concourse.mybir as mybir
from concourse.tile import TileContext
from concourse.bass_utils import run_bass_kernel_spmd

BF16 = mybir.dt.bfloat16
F32 = mybir.dt.float32
AF = mybir.ActivationFunctionType
ALU = mybir.AluOpType

P = 128
D = 2048  # model dims
T = 2048  # tokens per shard
ND = D // P  # 16 d-chunks
NPF = 16  # packed-frequency chunks
TS = 256  # tokens per slab
NSLAB = T // TS  # 8
NB = 4  # batch
NS = 4096  # full seq

bf16 = ml_dtypes.bfloat16

_CACHE = {}


def _build_nc(reps: int = 1):
    nc = bacc.Bacc("TRN2", target_bir_lowering=False, debug=False, num_devices=8)
    xS = nc.dram_tensor("xS", [NSLAB, P, ND, TS], BF16, kind="ExternalInput")
    CS = nc.dram_tensor("CS", [P, ND, D], BF16, kind="ExternalInput")
    M = nc.dram_tensor("M", [P, NPF, D], BF16, kind="ExternalInput")
    C0 = nc.dram_tensor("C0", [P, NPF], F32, kind="ExternalInput")
    ZR = nc.dram_tensor("ZR", [P, TS], BF16, kind="ExternalInput")
    out = nc.dram_tensor("out", [T, D], F32, kind="ExternalOutput")

    with TileContext(nc) as tc:
        with tc.tile_pool(name="misc", bufs=1) as misc:
            c0_sb = misc.tile([P, NPF], F32)
            nc.sync.dma_start(c0_sb[:], C0[:])
            zr_sb = misc.tile([P, TS], BF16)
            nc.sync.dma_start(zr_sb[:], ZR[:])
            # weights stay resident across repeat-loop iterations
            cs_sb = misc.tile([P, ND, D], BF16)
            nc.sync.dma_start(cs_sb[:], CS[:])
            m_sb = misc.tile([P, NPF, D], BF16)
            nc.sync.dma_start(m_sb[:], M[:])

            import contextlib

            loop_ctx = (
                tc.For_i(0, reps, 1) if reps > 1 else contextlib.nullcontext()
            )
            with loop_ctx:
                _body(nc, tc, c0_sb, zr_sb, cs_sb, m_sb, xS, out)
    nc.finalize()
    return nc


def _body(nc, tc, c0_sb, zr_sb, cs_sb, m_sb, xS, out):
    with (
        tc.tile_pool(name="xt", bufs=2) as xpool,
        tc.tile_pool(name="ss", bufs=2) as spool,
        tc.tile_pool(name="tmp", bufs=1) as tpool,
        tc.tile_pool(name="qvp", bufs=2) as qvpool,
        tc.tile_pool(name="osb", bufs=2) as opool,
        tc.tile_pool(name="psA", bufs=4, space="PSUM") as psumA,
        tc.tile_pool(name="psB", bufs=4, space="PSUM") as psumB,
    ):
        def emit_B_gen(qv_s, s):
            """Output matmul for one 256-token slab, yielded stepwise so it
            can be interleaved into PE gaps between the next slab's DFT
            passes."""
            for tsub in range(2):
                ob = opool.tile([P, D], F32, tag="osb")
                for e in range(4):
                    psb = psumB.tile([P, 512], F32, tag="psB")
                    for pf in range(NPF):
                        nc.tensor.matmul(
                            psb[:],
                            qv_s[:, pf, tsub * P : (tsub + 1) * P],
                            m_sb[:, pf, e * 512 : (e + 1) * 512],
                            start=(pf == 0),
                            stop=(pf == NPF - 1),
                        )
                        yield
                    if e % 2 == 0:
                        nc.scalar.copy(ob[:, e * 512 : (e + 1) * 512], psb[:])
                    else:
                        nc.vector.tensor_copy(
                            ob[:, e * 512 : (e + 1) * 512], psb[:]
                        )
                    yield
                r0 = s * TS + tsub * P
                nc.sync.dma_start(out[r0 : r0 + P, :], ob[:])
                yield

        def adv(gen, n):
            if gen is None:
                return
            for _ in range(n):
                if next(gen, "done") == "done":
                    return

        S_prev = None
        bgen = None
        for s in range(NSLAB):
            xt = xpool.tile([P, ND, TS], BF16, tag="xt")
            nc.sync.dma_start(xt[:], xS[s])
            S_sb = spool.tile([P, NPF, TS], BF16, tag="S")
            qv = qvpool.tile([P, NPF, TS], BF16, tag="qv")
            for p4 in range(4):
                # pass covers a Re chunk pair and its Im partner pair; each
                # frequency chunk accumulates in its OWN psum bank (an
                # accumulation group's start clears has_written bank-wide,
                # so groups must not share banks)
                fcs = [2 * p4, 2 * p4 + 1, 8 + 2 * p4, 8 + 2 * p4 + 1]
                ps4 = [
                    psumA.tile([P, TS], F32, tag="psA", name=f"ps{p4}_{j}")
                    for j in range(4)
                ]
                for d in range(ND):
                    for j, fc in enumerate(fcs):
                        nc.tensor.matmul(
                            ps4[j][:],
                            cs_sb[:, d, fc * P : (fc + 1) * P],
                            xt[:, d, :],
                            start=(d == 0),
                            stop=(d == ND - 1),
                        )
                # previous slab's output matmul fills the PE pipeline while
                # this pass's spectrum is scanned/multiplied
                adv(bgen, 34)

                for j, fc in enumerate(fcs):
                    init = (
                        c0_sb[:, fc : fc + 1]
                        if s == 0
                        else S_prev[:, fc, TS - 1 : TS]
                    )
                    nc.vector.tensor_tensor_scan(
                        S_sb[:, fc, :],
                        ps4[j][:],
                        zr_sb[:],
                        initial=init,
                        op0=ALU.add,
                        op1=ALU.add,
                    )
                for i in range(2):  # the two complex pairs of this pass
                    fre, fim = 2 * p4 + i, 8 + 2 * p4 + i
                    qre, qim = ps4[i], ps4[2 + i]
                    sre, sim_ = S_sb[:, fre, :], S_sb[:, fim, :]
                    t1 = tpool.tile([P, TS], F32, tag="t1")
                    t2 = tpool.tile([P, TS], F32, tag="t2")
                    nc.vector.tensor_mul(t1[:], qre[:], sre)
                    nc.vector.tensor_mul(t2[:], qim[:], sim_)
                    nc.vector.tensor_sub(qv[:, fre, :], t1[:], t2[:])
                    t3 = tpool.tile([P, TS], F32, tag="t1")
                    t4 = tpool.tile([P, TS], F32, tag="t2")
                    nc.vector.tensor_mul(t3[:], qre[:], sim_)
                    nc.vector.tensor_mul(t4[:], qim[:], sre)
                    nc.vector.tensor_add(qv[:, fim, :], t3[:], t4[:])
                    if p4 == 0 and i == 0:
                        # DC (chunk 0 row 0) and Nyquist (chunk 8 row 0):
                        # purely real
                        nc.vector.tensor_mul(
                            qv[0:1, 0, :], qre[0:1, :], S_sb[0:1, 0, :]
                        )
                        nc.vector.tensor_mul(
                            qv[0:1, 8, :], qim[0:1, :], S_sb[0:1, 8, :]
                        )
            S_prev = S_sb

            # drain the rest of the previous slab's output matmul
            adv(bgen, 200)
            bgen = emit_B_gen(qv, s)

        adv(bgen, 200)


def _chunked(m):
    """[rows, cols] -> [P, rows//P, cols] with row r at [r % P, r // P]."""
    r, c = m.shape
    return np.ascontiguousarray(m.reshape(r // P, P, c).transpose(1, 0, 2))


def _pack_spec(re, im):
    """re[1025], im[1025] -> packed [2048]: re[0..1024] then im[1..1023]."""
    return np.concatenate([re, im[1:1024]])


def _constants():
    if "consts" in _CACHE:
        return _CACHE["consts"]
    d = np.arange(D, dtype=np.float64)
    f = np.arange(D // 2 + 1, dtype=np.float64)
    ang = 2.0 * np.pi / D * np.outer(d, f)  # [D, 1025]
    cos, sin = np.cos(ang), np.sin(ang)
    CSf = np.concatenate([cos, -sin[:, 1:1024]], axis=1)  # [D, D]
    alpha = np.full(1025, 2.0)
    alpha[0] = alpha[1024] = 1.0
    Gf = np.concatenate(
        [(alpha[:, None] * cos.T) / D, (-2.0 * sin[:, 1:1024].T) / D], axis=0
    )  # [D packed, D]
    consts = {
        "CS": _chunked(CSf.astype(np.float32)).astype(bf16),
        "Gf32": Gf.astype(np.float32),
    }
    _CACHE["consts"] = consts
    return consts


def prepare_in_maps(x, queries, keyvalues, w_out):
    x = np.asarray(x, dtype=np.float32)
    queries = np.asarray(queries, dtype=np.float32)
    keyvalues = np.asarray(keyvalues, dtype=np.float32)
    w_out = np.asarray(w_out, dtype=np.float32)
    consts = _constants()

    c = (queries * keyvalues).reshape(-1)  # [1025]
    cpk = _pack_spec(c, c).astype(np.float32)  # [2048]
    # irfft + output Linear + c-filter as ONE matrix: M = diag(c_pk) G w_out^T
    Mfull = (consts["Gf32"] * cpk[:, None]) @ np.ascontiguousarray(w_out.T)
    Mc = _chunked(Mfull).astype(bf16)
    zr = np.zeros((P, TS), bf16)

    in_maps = []
    shards = []
    for b in range(NB):
        for h in range(2):
            shards.append((b, h))
            xs = x[b, h * T : (h + 1) * T]  # [T, D]
            xT3 = _chunked(np.ascontiguousarray(xs.T))  # [P, ND, T]
            xSc = np.ascontiguousarray(
                xT3.reshape(P, ND, NSLAB, TS).transpose(2, 0, 1, 3)
            ).astype(bf16)
            if h == 0:
                c0 = np.zeros((P, NPF), np.float32)
            else:
                F = np.fft.rfft(x[b, :T].sum(axis=0).astype(np.float64))
                c0 = _chunked(
                    _pack_spec(F.real, F.imag).astype(np.float32)[:, None]
                )[:, :, 0]
            in_maps.append(
                {
                    "xS": xSc,
                    "CS": consts["CS"],
                    "M": Mc,
                    "C0": np.ascontiguousarray(c0),
                    "ZR": zr,
                }
            )
    return in_maps, shards


def kernel(x, queries, keyvalues, w_out):
    if "nc" not in _CACHE:
        _CACHE["nc"] = _build_nc()
    nc = _CACHE["nc"]
    in_maps, shards = prepare_in_maps(x, queries, keyvalues, w_out)
    res = run_bass_kernel_spmd(nc, in_maps, core_ids=list(range(8)))
    y = np.empty((NB, NS, D), np.float32)
    for i, (b, h) in enumerate(shards):
        y[b, h * T : (h + 1) * T] = res.results[i]["out"]
    return y


# revision 11
# speedup vs baseline: 1.3583x; 1.2952x over previous
"""HRR binding self-attention kernel for 8 trn2 NeuronCores.

Math: out = irfft(c * rfft(x) * cumsum_s(rfft(x))) @ w_out.T  with c = queries*keyvalues.
Since rfft is linear, cumsum commutes with it: only ONE forward DFT of x is needed;
the causal prefix sum runs in the frequency domain.  Two further fusions:
  * irfft followed by the output Linear is one linear map:  out = qv^T (G @ w_out.T),
    precomputed on host as M (packed-spectrum x model_dims).
  * the real per-frequency filter c is diagonal in the packed spectrum, so it folds
    into M as a row scale:  M_c = diag(c_packed) G w_out^T.
So the device does: DFT (matmul, emitted FREQ-major so the spectrum lands in PSUM
with frequency on partitions), causal prefix sum via the DVE's tensor_tensor_scan
(per-partition recurrence along tokens, carry chained through `initial`), complex
pointwise multiply reading Q directly from PSUM, and ONE output matmul with M_c.
No transpose stage, no PSUM->SBUF spectrum eviction, and only ~24 DMAs/iteration.

Sharding: 8 shards = (batch b in 0..3) x (seq half h in 0..1), 2048 tokens each.
The h=1 shards get the first half's contribution as an initial carry, computed on
host as rfft(x[b, :2048].sum(0)) (O(B*D log D) -- negligible).

Packed real spectrum (2048 rows): rows 0..1024 = Re[0..1024], rows 1025..2047 =
Im[1..1023].  Row 1024 (Nyquist, purely real) rides in the Im-block's first slot
(chunk 8, partition 0); complex multiplies pair chunk i with chunk 8+i on equal
partitions, with a 2-row fixup for the DC/Nyquist slots.

Per-core pipeline over 256-token slabs (all matmuls bf16, fp32 PSUM):
  4 DFT passes per slab, each producing 4 freq-chunks {2p, 2p+1, 2p+8, 2p+9}
  (a Re pair and its Im partner pair) into 2 PSUM banks; scan -> S; 6 DVE ops
  -> qv;  the PREVIOUS slab's output matmul (qv chunk^T @ M_c) is interleaved
  between DFT passes so the PE never idles.
"""

import sys

sys.path.insert(0, "/opt/trn_rl_repo")

import numpy as np
import ml_dtypes

import concourse.bass as bass
import concourse.bacc as bacc
import concourse.mybir as mybir
from concourse.tile import TileContext
from concourse.bass_utils import run_bass_kernel_spmd

BF16 = mybir.dt.bfloat16
F32 = mybir.dt.float32
AF = mybir.ActivationFunctionType
ALU = mybir.AluOpType

P = 128
D = 2048  # model dims
T = 2048  # tokens per shard
ND = D // P  # 16 d-chunks
NPF = 16  # packed-frequency chunks
TS = 256  # tokens per slab
NSLAB = T // TS  # 8
NB = 4  # batch
NS = 4096  # full seq

bf16 = ml_dtypes.bfloat16

_CACHE = {}


def _build_nc(reps: int = 1):
    nc = bacc.Bacc("TRN2", target_bir_lowering=False, debug=False, num_devices=8)
    xS = nc.dram_tensor("xS", [NSLAB, P, ND, TS], BF16, kind="ExternalInput")
    CS = nc.dram_tensor("CS", [P, ND, D], BF16, kind="ExternalInput")
    M = nc.dram_tensor("M", [P, NPF, D], BF16, kind="ExternalInput")
    C0 = nc.dram_tensor("C0", [P, NPF], F32, kind="ExternalInput")
    ZR = nc.dram_tensor("ZR", [P, TS], BF16, kind="ExternalInput")
    out = nc.dram_tensor("out", [T, D], F32, kind="ExternalOutput")

    with TileContext(nc) as tc:
        with tc.tile_pool(name="misc", bufs=1) as misc:
            c0_sb = misc.tile([P, NPF], F32)
            nc.sync.dma_start(c0_sb[:], C0[:])
            zr_sb = misc.tile([P, TS], BF16)
            nc.sync.dma_start(zr_sb[:], ZR[:])
            # weights stay resident across repeat-loop iterations
            cs_sb = misc.tile([P, ND, D], BF16)
            nc.sync.dma_start(cs_sb[:], CS[:])
            m_sb = misc.tile([P, NPF, D], BF16)
            nc.sync.dma_start(m_sb[:], M[:])

            import contextlib

            loop_ctx = (
                tc.For_i(0, reps, 1) if reps > 1 else contextlib.nullcontext()
            )
            with loop_ctx:
                _body(nc, tc, c0_sb, zr_sb, cs_sb, m_sb, xS, out)
    nc.finalize()
    return nc


def _body(nc, tc, c0_sb, zr_sb, cs_sb, m_sb, xS, out):
    with (
        tc.tile_pool(name="xt", bufs=2) as xpool,
        tc.tile_pool(name="ss", bufs=2) as spool,
        tc.tile_pool(name="tmp", bufs=1) as tpool,
        tc.tile_pool(name="qvp", bufs=2) as qvpool,
        tc.tile_pool(name="osb", bufs=2) as opool,
        tc.tile_pool(name="psA", bufs=4, space="PSUM") as psumA,
        tc.tile_pool(name="psB", bufs=4, space="PSUM") as psumB,
    ):
        def emit_B_gen(qv_s, s):
            """Output matmul for one 256-token slab, yielded stepwise so it
            can be interleaved into PE gaps between the next slab's DFT
            passes.  pf-outer order: each 128-token qv chunk is loaded as
            stationary weight ONCE and streamed against all four 512-wide
            M column blocks (4 PSUM banks accumulate in parallel), instead
            of reloading the weight per block."""
            for tsub in range(2):
                ob = opool.tile([P, D], F32, tag="osb")
                psbs = [
                    psumB.tile([P, 512], F32, tag="psB", name=f"psb{e}")
                    for e in range(4)
                ]
                for pf in range(NPF):
                    for e in range(4):
                        nc.tensor.matmul(
                            psbs[e][:],
                            qv_s[:, pf, tsub * P : (tsub + 1) * P],
                            m_sb[:, pf, e * 512 : (e + 1) * 512],
                            start=(pf == 0),
                            stop=(pf == NPF - 1),
                        )
                        yield
                for e in range(4):
                    if e % 2 == 0:
                        nc.scalar.copy(ob[:, e * 512 : (e + 1) * 512], psbs[e][:])
                    else:
                        nc.vector.tensor_copy(
                            ob[:, e * 512 : (e + 1) * 512], psbs[e][:]
                        )
                    yield
                r0 = s * TS + tsub * P
                nc.sync.dma_start(out[r0 : r0 + P, :], ob[:])
                yield

        def adv(gen, n):
            if gen is None:
                return
            for _ in range(n):
                if next(gen, "done") == "done":
                    return

        S_prev = None
        bgen = None
        for s in range(NSLAB):
            xt = xpool.tile([P, ND, TS], BF16, tag="xt")
            nc.sync.dma_start(xt[:], xS[s])
            S_sb = spool.tile([P, NPF, TS], BF16, tag="S")
            qv = qvpool.tile([P, NPF, TS], BF16, tag="qv")
            for p4 in range(4):
                # pass covers a Re chunk pair and its Im partner pair; each
                # frequency chunk accumulates in its OWN psum bank (an
                # accumulation group's start clears has_written bank-wide,
                # so groups must not share banks)
                fcs = [2 * p4, 2 * p4 + 1, 8 + 2 * p4, 8 + 2 * p4 + 1]
                ps4 = [
                    psumA.tile([P, TS], F32, tag="psA", name=f"ps{p4}_{j}")
                    for j in range(4)
                ]
                for d in range(ND):
                    for j, fc in enumerate(fcs):
                        nc.tensor.matmul(
                            ps4[j][:],
                            cs_sb[:, d, fc * P : (fc + 1) * P],
                            xt[:, d, :],
                            start=(d == 0),
                            stop=(d == ND - 1),
                        )
                # previous slab's output matmul fills the PE pipeline while
                # this pass's spectrum is scanned/multiplied
                adv(bgen, 34)

                for j, fc in enumerate(fcs):
                    init = (
                        c0_sb[:, fc : fc + 1]
                        if s == 0
                        else S_prev[:, fc, TS - 1 : TS]
                    )
                    nc.vector.tensor_tensor_scan(
                        S_sb[:, fc, :],
                        ps4[j][:],
                        zr_sb[:],
                        initial=init,
                        op0=ALU.add,
                        op1=ALU.add,
                    )
                for i in range(2):  # the two complex pairs of this pass
                    fre, fim = 2 * p4 + i, 8 + 2 * p4 + i
                    qre, qim = ps4[i], ps4[2 + i]
                    sre, sim_ = S_sb[:, fre, :], S_sb[:, fim, :]
                    t1 = tpool.tile([P, TS], F32, tag="t1")
                    t2 = tpool.tile([P, TS], F32, tag="t2")
                    nc.vector.tensor_mul(t1[:], qre[:], sre)
                    nc.vector.tensor_mul(t2[:], qim[:], sim_)
                    nc.vector.tensor_sub(qv[:, fre, :], t1[:], t2[:])
                    t3 = tpool.tile([P, TS], F32, tag="t1")
                    t4 = tpool.tile([P, TS], F32, tag="t2")
                    nc.vector.tensor_mul(t3[:], qre[:], sim_)
                    nc.vector.tensor_mul(t4[:], qim[:], sre)
                    nc.vector.tensor_add(qv[:, fim, :], t3[:], t4[:])
                    if p4 == 0 and i == 0:
                        # DC (chunk 0 row 0) and Nyquist (chunk 8 row 0):
                        # purely real
                        nc.vector.tensor_mul(
                            qv[0:1, 0, :], qre[0:1, :], S_sb[0:1, 0, :]
                        )
                        nc.vector.tensor_mul(
                            qv[0:1, 8, :], qim[0:1, :], S_sb[0:1, 8, :]
                        )
            S_prev = S_sb

            # drain the rest of the previous slab's output matmul
            adv(bgen, 200)
            bgen = emit_B_gen(qv, s)

        adv(bgen, 200)


def _chunked(m):
    """[rows, cols] -> [P, rows//P, cols] with row r at [r % P, r // P]."""
    r, c = m.shape
    return np.ascontiguousarray(m.reshape(r // P, P, c).transpose(1, 0, 2))


def _pack_spec(re, im):
    """re[1025], im[1025] -> packed [2048]: re[0..1024] then im[1..1023]."""
    return np.concatenate([re, im[1:1024]])


def _constants():
    if "consts" in _CACHE:
        return _CACHE["consts"]
    d = np.arange(D, dtype=np.float64)
    f = np.arange(D // 2 + 1, dtype=np.float64)
    ang = 2.0 * np.pi / D * np.outer(d, f)  # [D, 1025]
    cos, sin = np.cos(ang), np.sin(ang)
    CSf = np.concatenate([cos, -sin[:, 1:1024]], axis=1)  # [D, D]
    alpha = np.full(1025, 2.0)
    alpha[0] = alpha[1024] = 1.0
    Gf = np.concatenate(
        [(alpha[:, None] * cos.T) / D, (-2.0 * sin[:, 1:1024].T) / D], axis=0
    )  # [D packed, D]
    consts = {
        "CS": _chunked(CSf.astype(np.float32)).astype(bf16),
        "Gf32": Gf.astype(np.float32),
    }
    _CACHE["consts"] = consts
    return consts


def prepare_in_maps(x, queries, keyvalues, w_out):
    x = np.asarray(x, dtype=np.float32)
    queries = np.asarray(queries, dtype=np.float32)
    keyvalues = np.asarray(keyvalues, dtype=np.float32)
    w_out = np.asarray(w_out, dtype=np.float32)
    consts = _constants()

    c = (queries * keyvalues).reshape(-1)  # [1025]
    cpk = _pack_spec(c, c).astype(np.float32)  # [2048]
    # irfft + output Linear + c-filter as ONE matrix: M = diag(c_pk) G w_out^T
    Mfull = (consts["Gf32"] * cpk[:, None]) @ np.ascontiguousarray(w_out.T)
    Mc = _chunked(Mfull).astype(bf16)
    zr = np.zeros((P, TS), bf16)

    in_maps = []
    shards = []
    for b in range(NB):
        for h in range(2):
            shards.append((b, h))
            xs = x[b, h * T : (h + 1) * T]  # [T, D]
            xT3 = _chunked(np.ascontiguousarray(xs.T))  # [P, ND, T]
            xSc = np.ascontiguousarray(
                xT3.reshape(P, ND, NSLAB, TS).transpose(2, 0, 1, 3)
            ).astype(bf16)
            if h == 0:
                c0 = np.zeros((P, NPF), np.float32)
            else:
                F = np.fft.rfft(x[b, :T].sum(axis=0).astype(np.float64))
                c0 = _chunked(
                    _pack_spec(F.real, F.imag).astype(np.float32)[:, None]
                )[:, :, 0]
            in_maps.append(
                {
                    "xS": xSc,
                    "CS": consts["CS"],
                    "M": Mc,
                    "C0": np.ascontiguousarray(c0),
                    "ZR": zr,
                }
            )
    return in_maps, shards


def kernel(x, queries, keyvalues, w_out):
    if "nc" not in _CACHE:
        _CACHE["nc"] = _build_nc()
    nc = _CACHE["nc"]
    in_maps, shards = prepare_in_maps(x, queries, keyvalues, w_out)
    res = run_bass_kernel_spmd(nc, in_maps, core_ids=list(range(8)))
    y = np.empty((NB, NS, D), np.float32)
    for i, (b, h) in enumerate(shards):
        y[b, h * T : (h + 1) * T] = res.results[i]["out"]
    return y
